# revision 1
# baseline (speedup 1.0000x reference)
"""Trainium2 Bass kernel for nn_DetectionLoss (SSD-style detection loss).

Strategy: data-parallel over the batch — 16 images, 8 NeuronCores, 2 images
per core. Each core computes, per image: the dense [A, G] anchor/gt IoU grid
(anchors on partitions, gt along the free dim, supertiles of 16 anchor
columns x 32 gts = 512-wide vector ops), per-anchor best-gt matching
(max + is_equal one-hot, summed against gt coords), per-gt best-anchor
forcing (column max via PE transpose + second eq pass over the SBUF-resident
grid), DIoU loc loss, sigmoid focal conf loss, and hard-negative mining via
a two-level (64x64) threshold-count search that returns the exact
sum-above-threshold plus boundary-bin statistics. The host combines the
per-image scalars (sum / max(1, n_pos)) exactly as the reference does.
"""
import sys

sys.path.insert(0, '/opt/trn_rl_repo')

import numpy as np
import concourse.bass as bass
import concourse.bacc as bacc
import concourse.mybir as mybir
from concourse.tile import TileContext
from concourse.bass_utils import run_bass_kernel_spmd
from concourse.masks import make_identity
from contextlib import ExitStack

Alu = mybir.AluOpType
Act = mybir.ActivationFunctionType
Ax = mybir.AxisListType
F32 = mybir.dt.float32
I32 = mybir.dt.int32

P = 128
A = 65536
G = 32
IMG = 2            # images per core
NCORE = 8
COLS = A // P      # 512 anchor columns per partition
U = 32             # anchor columns per supertile
W = U * G          # 1024 free elems per supertile
NSUP = COLS // U   # 32 supertiles
EPS = 1e-7
NBIN = 16          # histogram bins per mining level
NLEV = 3           # mining levels (resolution NBIN**NLEV = 4096)
NEG_POS_RATIO = 3.0


def _build_nc():
    nc = bacc.Bacc("TRN2", target_bir_lowering=False, debug=False)
    anch_d = nc.dram_tensor("anch", [P, COLS * 4], F32, kind="ExternalInput")
    bbox_d = nc.dram_tensor("bbox", [IMG, P, COLS * 4], F32, kind="ExternalInput")
    conf_d = nc.dram_tensor("conf", [IMG, P, COLS], F32, kind="ExternalInput")
    gt_d = nc.dram_tensor("gtb", [IMG, 1, G * 4], F32, kind="ExternalInput")
    res_d = nc.dram_tensor("res", [IMG, 1, 8], F32, kind="ExternalOutput")

    v = nc.vector
    sc = nc.scalar
    pe = nc.tensor

    with TileContext(nc) as tc, ExitStack() as ctx:
        pool = ctx.enter_context(tc.tile_pool(name="main", bufs=1))
        pspool = ctx.enter_context(tc.tile_pool(name="ps", bufs=1, space="PSUM"))

        def T(name, cols, parts=P):
            return pool.tile([parts, cols], F32, name=name)

        # ---------------- static tiles ----------------
        anch_sb = T("anch_sb", COLS * 4)        # 8 KB/part
        grid = T("grid", COLS * G)              # 64 KB/part
        bbox_sb = T("bbox_sb", COLS * 4)        # 8 KB
        conf_sb = T("conf_sb", COLS)
        areaAe = T("areaAe", COLS)
        rowmax = T("rowmax", COLS)
        forced = T("forced", COLS)
        pos = T("pos", COLS)
        matched_il = T("matched_il", COLS * 4)  # 8 KB, (col, coord) interleaved
        colparts = T("colparts", NSUP * G)
        nv = [T(f"nv{i}", COLS) for i in range(IMG)]
        cl = T("cl", COLS)
        sink = T("sink", COLS)
        BF16 = mybir.dt.bfloat16
        FP16 = mybir.dt.float16
        nv16 = [pool.tile([P, COLS], FP16, name=f"nv16_{i}") for i in range(IMG)]
        sink16 = pool.tile([P, COLS], FP16, name="sink16")
        fb16 = pool.tile([P, W], BF16, name="fb16")
        # grid-phase scratch
        lt = T("lt", 2 * W)                     # 8 KB
        rb = T("rb", 2 * W)                     # 8 KB
        inter = T("inter", W)
        csum = T("csum", W)
        rec = T("rec", W)
        ismax = T("ismax", W)
        tsb = [T(f"tsb{i}", P) for i in range(2)]  # transposed ismax chunks
        gtmat = T("gtmat", 16)                  # block-diag gt coords [128,(uu,c)]
        # wide scratch (diou/focal), reused heavily
        w0 = T("w0", 2 * COLS)
        w1_ = T("w1_", 2 * COLS)
        s0 = T("s0", COLS)
        s1 = T("s1", COLS)
        s2 = T("s2", COLS)
        s3 = T("s3", COLS)
        s4 = T("s4", COLS)
        s5 = T("s5", COLS)
        # gt tiles
        gtc = [T(f"gtc{c}", G) for c in range(4)]
        sG = T("sG", G)
        glo = T("glo", 2 * G)
        ghi = T("ghi", 2 * G)
        cmb = T("cmb", G)
        colT = T("colT", G)
        ctt = T("ctt", P, parts=G)
        cmax_col = T("cmax_col", 1, parts=G)
        cm_row = T("cm_row", G, parts=1)
        mx_row = T("mx_row", P, parts=1)
        # small column tiles
        npp = T("npp", 1)
        locsum_pp = T("locsum_pp", 1)
        possum_pp = T("possum_pp", 1)
        cnt_pp = T("cnt_pp", 1)
        sum_pp = T("sum_pp", 1)
        maxv_pp = T("maxv_pp", 1)
        maxvb = T("maxvb", 1)
        w1c = T("w1c", 1)
        tau_b = T("tau_b", 1)
        stack = T("stack", 4)
        ones_col = T("ones_col", 1)
        ones_row = T("ones_row", P, parts=1)
        ident = T("ident", P)
        iota_f = T("iota_f", NBIN)
        thr = T("thr", NBIN)
        cge = T("cge", NBIN)
        wl = [T(f"wl{l}", 1) for l in range(NLEV)]
        lo_b = [T(f"lo_b{l}", 1) for l in range(NLEV)]
        cget = T("cget", NBIN, parts=1)
        gek = T("gek", NBIN, parts=1)
        scnt = T("scnt", 1, parts=1)
        lo_new = T("lo_new", 1, parts=1)
        tau = [T(f"tau{l}", 1, parts=1) for l in range(NLEV)]
        maxv1 = T("maxv1", 1, parts=1)
        npos1 = T("npos1", 1, parts=1)
        k1 = T("k1", 1, parts=1)
        k2 = T("k2", 1, parts=1)
        kk = T("kk", 1, parts=1)
        res_sb = T("res_sb", 8, parts=1)
        iota_i = pool.tile([P, NBIN], I32, name="iota_i")

        # ---------------- constants ----------------
        nc.sync.dma_start(anch_sb[:], anch_d[:])
        anch3 = anch_sb[:].rearrange("p (n c) -> p n c", c=4)
        v.tensor_tensor(out=s0[:], in0=anch3[:, :, 2:3].squeeze(2),
                        in1=anch3[:, :, 0:1].squeeze(2), op=Alu.subtract)
        v.tensor_tensor(out=s1[:], in0=anch3[:, :, 3:4].squeeze(2),
                        in1=anch3[:, :, 1:2].squeeze(2), op=Alu.subtract)
        v.tensor_tensor(out=areaAe[:], in0=s0[:], in1=s1[:], op=Alu.mult)
        v.tensor_scalar(areaAe[:], areaAe[:], float(EPS), None, Alu.add)

        v.memset(ones_col[:], 1.0)
        v.memset(ones_row[:], 1.0)
        make_identity(nc, ident[:])

        def pbcast(dst, src_row):
            """Broadcast a [1, n] partition-0 row to [P, n] via a K=1 matmul."""
            n = src_row.shape[-1]
            bc_ps = pspool.tile([P, G], F32, name="bc_ps", tag="pss")
            nc.tensor.matmul(bc_ps[:, 0:n], ones_row[:], src_row)
            v.tensor_copy(dst, bc_ps[:, 0:n])
        nc.gpsimd.iota(iota_i[:], pattern=[[1, NBIN]], base=0, channel_multiplier=0)
        v.tensor_copy(iota_f[:], iota_i[:])

        for b in range(IMG):
            # ---------------- per-image loads ----------------
            nc.sync.dma_start(bbox_sb[:], bbox_d[b])
            nc.sync.dma_start(conf_sb[:], conf_d[b])
            bb3 = bbox_sb[:].rearrange("p (n c) -> p n c", c=4)

            gt3 = gt_d[b].rearrange("q (g c) -> q g c", c=4)  # [1, 32, 4] dram
            for c in range(4):
                nc.sync.dma_start(gtc[c][:],
                                  gt3[:, :, c:c+1].squeeze(2).partition_broadcast(P))
            # block-diagonal gt-coordinate matrix for the matched-gt matmul:
            # gtmat[uu*32+g, uu*4+c] = gt[g, c]
            v.memset(gtmat[:], 0.0)
            gt2d = gt_d[b].rearrange("q (g c) -> (q g) c", c=4)  # [32, 4] dram
            for uu in range(4):
                nc.sync.dma_start(gtmat[uu * G:(uu + 1) * G, uu * 4:(uu + 1) * 4],
                                  gt2d)
            v.tensor_tensor(out=glo[:, 0:G], in0=gtc[2][:], in1=gtc[0][:], op=Alu.subtract)
            v.tensor_tensor(out=glo[:, G:], in0=gtc[3][:], in1=gtc[1][:], op=Alu.subtract)
            v.tensor_tensor(out=sG[:], in0=glo[:, 0:G], in1=glo[:, G:], op=Alu.mult)
            v.tensor_copy(glo[:, 0:G], gtc[0][:])
            v.tensor_copy(glo[:, G:], gtc[1][:])
            v.tensor_copy(ghi[:, 0:G], gtc[2][:])
            v.tensor_copy(ghi[:, G:], gtc[3][:])

            def gb(t):  # [P, G] tile -> broadcast [P, U, G]
                return t[:].unsqueeze(1).to_broadcast([P, U, G])

            # ---------------- grid phase ----------------
            for s in range(NSUP):
                csl = slice(s * U, (s + 1) * U)
                lt4 = lt[:].rearrange("p (u c g) -> p u c g", c=2, g=G)
                rb4 = rb[:].rearrange("p (u c g) -> p u c g", c=2, g=G)
                a_lo = anch3[:, csl, 0:2].unsqueeze(3).to_broadcast([P, U, 2, G])
                a_hi = anch3[:, csl, 2:4].unsqueeze(3).to_broadcast([P, U, 2, G])
                g_lo = glo[:].rearrange("p (c g) -> p c g", g=G).unsqueeze(1).to_broadcast([P, U, 2, G])
                g_hi = ghi[:].rearrange("p (c g) -> p c g", g=G).unsqueeze(1).to_broadcast([P, U, 2, G])
                v.tensor_tensor(out=lt4, in0=a_lo, in1=g_lo, op=Alu.max)
                v.tensor_tensor(out=rb4, in0=a_hi, in1=g_hi, op=Alu.min)
                v.tensor_tensor(out=lt[:], in0=rb[:], in1=lt[:], op=Alu.subtract)
                sc.activation(lt[:], lt[:], Act.Relu)
                wr4 = lt[:].rearrange("p (u c g) -> p u c g", c=2, g=G)

                inter3 = inter[:].rearrange("p (u g) -> p u g", g=G)
                v.tensor_tensor(out=inter3, in0=wr4[:, :, 0, :], in1=wr4[:, :, 1, :],
                                op=Alu.mult)
                csum3 = csum[:].rearrange("p (u g) -> p u g", g=G)
                v.tensor_tensor(out=csum3,
                                in0=areaAe[:, csl].unsqueeze(2).to_broadcast([P, U, G]),
                                in1=gb(sG), op=Alu.add)
                v.tensor_tensor(out=csum[:], in0=csum[:], in1=inter[:], op=Alu.subtract)
                v.reciprocal_approx_fast(out=rec[:], in_=csum[:])

                gsl = grid[:, s * W:(s + 1) * W]
                g3 = gsl.rearrange("p (u g) -> p u g", g=G)
                v.tensor_tensor(out=g3, in0=inter[:].rearrange("p (u g) -> p u g", g=G),
                                in1=rec[:].rearrange("p (u g) -> p u g", g=G), op=Alu.mult)
                v.tensor_reduce(out=rowmax[:, csl], in_=g3, axis=Ax.X, op=Alu.max)
                ismax3 = ismax[:].rearrange("p (u g) -> p u g", g=G)
                v.tensor_tensor(out=ismax3, in0=g3,
                                in1=rowmax[:, csl].unsqueeze(2).to_broadcast([P, U, G]),
                                op=Alu.is_equal)
                # matched gt coords via PE: transpose each [128,128] ismax chunk,
                # then contract over gt with the block-diagonal gtmat.
                mout = pspool.tile([P, P], F32, name="mout", tag="mout")
                for j in range(W // P):
                    tp = pspool.tile([P, P], F32, name=f"tp{j % 2}", tag=f"tp{j % 2}")
                    pe.transpose(tp[:], ismax[:, j * P:(j + 1) * P], ident[:])
                    sc.copy(tsb[j % 2][:], tp[:])
                    nc.tensor.matmul(mout[:, j * 16:(j + 1) * 16], tsb[j % 2][:],
                                     gtmat[:])
                sc.copy(matched_il[:, s * (U * 4):(s + 1) * (U * 4)], mout[:])
                g3r = gsl.rearrange("p (u g) -> p g u", g=G)
                v.tensor_reduce(out=colparts[:, s * G:(s + 1) * G], in_=g3r,
                                axis=Ax.X, op=Alu.max)

            # ---------------- column max finalize ----------------
            v.tensor_reduce(out=colT[:],
                            in_=colparts[:].rearrange("p (s g) -> p g s", g=G),
                            axis=Ax.X, op=Alu.max)
            ct_ps = pspool.tile([G, P], F32, name="ct_ps", tag="pss")
            pe.transpose(ct_ps[:], colT[:], ident[:])
            v.tensor_copy(ctt[:], ct_ps[:])
            v.tensor_reduce(out=cmax_col[:], in_=ctt[:], axis=Ax.X, op=Alu.max)
            cm_ps = pspool.tile([1, G], F32, name="cm_ps", tag="pss")
            pe.transpose(cm_ps[:], cmax_col[:], ident[:G, :G])
            v.tensor_copy(cm_row[:], cm_ps[:])
            pbcast(cmb[:], cm_row[:])

            # ---------------- forced pass ----------------
            for s in range(NSUP):
                csl = slice(s * U, (s + 1) * U)
                g3 = grid[:, s * W:(s + 1) * W].rearrange("p (u g) -> p u g", g=G)
                fb3 = fb16[:].rearrange("p (u g) -> p u g", g=G)
                v.tensor_tensor(out=fb3, in0=g3, in1=gb(cmb), op=Alu.is_equal)
                v.tensor_reduce(out=forced[:, csl], in_=fb3, axis=Ax.X, op=Alu.max)

            # ---------------- pos / n_pos ----------------
            v.tensor_scalar(pos[:], rowmax[:], 0.5, None, Alu.is_gt)
            v.tensor_tensor(out=pos[:], in0=pos[:], in1=forced[:], op=Alu.max)
            v.tensor_reduce(out=npp[:], in_=pos[:], axis=Ax.X, op=Alu.add)

            # ---------------- DIoU loc loss ----------------
            def bc(c):
                return bb3[:, :, c:c+1].squeeze(2)

            m3 = matched_il[:].rearrange("p (n c) -> p n c", c=4)

            def mc(c):
                return m3[:, :, c:c+1].squeeze(2)

            # w0 = lt(x|y), w1_ = rb(x|y)
            v.tensor_tensor(out=w0[:, 0:COLS], in0=bc(0), in1=mc(0), op=Alu.max)
            v.tensor_tensor(out=w0[:, COLS:], in0=bc(1), in1=mc(1), op=Alu.max)
            v.tensor_tensor(out=w1_[:, 0:COLS], in0=bc(2), in1=mc(2), op=Alu.min)
            v.tensor_tensor(out=w1_[:, COLS:], in0=bc(3), in1=mc(3), op=Alu.min)
            v.tensor_tensor(out=w0[:], in0=w1_[:], in1=w0[:], op=Alu.subtract)
            sc.activation(w0[:], w0[:], Act.Relu)
            v.tensor_tensor(out=s0[:], in0=w0[:, 0:COLS], in1=w0[:, COLS:], op=Alu.mult)
            # s0 = inter; areas -> s1 (pred), s2 (matched)
            v.tensor_tensor(out=w0[:, 0:COLS], in0=bc(2), in1=bc(0), op=Alu.subtract)
            v.tensor_tensor(out=w0[:, COLS:], in0=bc(3), in1=bc(1), op=Alu.subtract)
            v.tensor_tensor(out=s1[:], in0=w0[:, 0:COLS], in1=w0[:, COLS:], op=Alu.mult)
            v.tensor_tensor(out=w0[:, 0:COLS], in0=mc(2), in1=mc(0), op=Alu.subtract)
            v.tensor_tensor(out=w0[:, COLS:], in0=mc(3), in1=mc(1), op=Alu.subtract)
            v.tensor_tensor(out=s2[:], in0=w0[:, 0:COLS], in1=w0[:, COLS:], op=Alu.mult)
            v.tensor_tensor(out=s1[:], in0=s1[:], in1=s2[:], op=Alu.add)
            v.tensor_tensor(out=s1[:], in0=s1[:], in1=s0[:], op=Alu.subtract)
            v.tensor_scalar(s1[:], s1[:], float(EPS), None, Alu.add)
            v.reciprocal_approx_accurate(out=s2[:], in_=s1[:], scratch=s3[:])
            v.tensor_tensor(out=s0[:], in0=s0[:], in1=s2[:], op=Alu.mult)  # s0 = iou
            # enclosing box
            v.tensor_tensor(out=w0[:, 0:COLS], in0=bc(0), in1=mc(0), op=Alu.min)
            v.tensor_tensor(out=w0[:, COLS:], in0=bc(1), in1=mc(1), op=Alu.min)
            v.tensor_tensor(out=w1_[:, 0:COLS], in0=bc(2), in1=mc(2), op=Alu.max)
            v.tensor_tensor(out=w1_[:, COLS:], in0=bc(3), in1=mc(3), op=Alu.max)
            v.tensor_tensor(out=w0[:], in0=w1_[:], in1=w0[:], op=Alu.subtract)
            sc.activation(w0[:], w0[:], Act.Square)
            v.tensor_tensor(out=s1[:], in0=w0[:, 0:COLS], in1=w0[:, COLS:], op=Alu.add)
            v.tensor_scalar(s1[:], s1[:], float(EPS), None, Alu.add)
            v.reciprocal_approx_accurate(out=s2[:], in_=s1[:], scratch=s3[:])  # 1/c2
            # center distance
            v.tensor_tensor(out=w0[:, 0:COLS], in0=bc(0), in1=bc(2), op=Alu.add)
            v.tensor_tensor(out=w0[:, COLS:], in0=bc(1), in1=bc(3), op=Alu.add)
            v.tensor_tensor(out=w1_[:, 0:COLS], in0=mc(0), in1=mc(2), op=Alu.add)
            v.tensor_tensor(out=w1_[:, COLS:], in0=mc(1), in1=mc(3), op=Alu.add)
            v.tensor_tensor(out=w0[:], in0=w0[:], in1=w1_[:], op=Alu.subtract)
            sc.activation(w0[:], w0[:], Act.Square, scale=0.5)
            v.tensor_tensor(out=s3[:], in0=w0[:, 0:COLS], in1=w0[:, COLS:], op=Alu.add)
            v.tensor_tensor(out=s3[:], in0=s3[:], in1=s2[:], op=Alu.mult)  # d2/c2
            v.tensor_scalar(s0[:], s0[:], -1.0, 1.0, Alu.mult, Alu.add)   # 1 - iou
            v.tensor_tensor(out=s3[:], in0=s3[:], in1=s0[:], op=Alu.add)
            v.tensor_scalar(s3[:], s3[:], 100.0, None, Alu.min)
            v.tensor_tensor(out=s4[:], in0=s3[:], in1=pos[:], op=Alu.mult)
            v.tensor_reduce(out=locsum_pp[:], in_=s4[:], axis=Ax.X, op=Alu.add)

            # ---------------- focal conf loss ----------------
            sc.activation(s0[:], conf_sb[:], Act.Sigmoid)
            sc.activation(s1[:], conf_sb[:], Act.Exp)
            sc.activation(s1[:], s1[:], Act.Ln, bias=1.0)
            v.tensor_tensor(out=s2[:], in0=conf_sb[:], in1=pos[:], op=Alu.mult)
            v.tensor_tensor(out=s2[:], in0=s1[:], in1=s2[:], op=Alu.subtract)  # ce
            v.tensor_scalar(s3[:], pos[:], -2.0, 1.0, Alu.mult, Alu.add)
            v.tensor_tensor(out=s3[:], in0=s0[:], in1=s3[:], op=Alu.mult)
            v.tensor_tensor(out=s3[:], in0=s3[:], in1=pos[:], op=Alu.add)  # 1-p_t
            sc.activation(s3[:], s3[:], Act.Square)
            v.tensor_tensor(out=cl[:], in0=s3[:], in1=s2[:], op=Alu.mult)
            v.tensor_scalar(s3[:], pos[:], -0.5, 0.75, Alu.mult, Alu.add)
            v.tensor_tensor(out=cl[:], in0=cl[:], in1=s3[:], op=Alu.mult)
            v.tensor_scalar(cl[:], cl[:], 100.0, None, Alu.min)
            v.tensor_tensor(out=s4[:], in0=cl[:], in1=pos[:], op=Alu.mult)
            v.tensor_reduce(out=possum_pp[:], in_=s4[:], axis=Ax.X, op=Alu.add)
            v.tensor_tensor(out=nv[b][:], in0=cl[:], in1=s4[:], op=Alu.subtract)
            v.tensor_copy(nv16[b][:], nv[b][:])

            # ---------------- hard negative mining ----------------
            v.tensor_reduce(out=maxv_pp[:], in_=nv[b][:], axis=Ax.X, op=Alu.max)
            mx_ps = pspool.tile([1, P], F32, name="mx_ps", tag="pss")
            pe.transpose(mx_ps[:], maxv_pp[:], ident[:])
            v.tensor_copy(mx_row[:], mx_ps[:])
            v.tensor_reduce(out=maxv1[:], in_=mx_row[:], axis=Ax.X, op=Alu.max)

            np_ps = pspool.tile([1, 1], F32, name="np_ps", tag="pss")
            nc.tensor.matmul(np_ps[:], ones_col[:], npp[:])
            v.tensor_copy(npos1[:], np_ps[:])
            v.tensor_scalar(k1[:], npos1[:], NEG_POS_RATIO, None, Alu.mult)
            v.tensor_scalar(k2[:], npos1[:], -1.0, float(A), Alu.mult, Alu.add)
            v.tensor_tensor(out=kk[:], in0=k1[:], in1=k2[:], op=Alu.min)

            pbcast(maxvb[:], maxv1[:])
            v.tensor_scalar(w1c[:], maxvb[:], 1.0 / NBIN, None, Alu.mult)

            for lev in range(NLEV):
                if lev == 0:
                    v.tensor_copy(wl[0][:], w1c[:])
                    v.tensor_scalar(thr[:], iota_f[:], wl[0][:], None, Alu.mult)
                else:
                    v.tensor_scalar(wl[lev][:], wl[lev - 1][:], 1.0 / NBIN, None,
                                    Alu.mult)
                    v.tensor_scalar(thr[:], iota_f[:], wl[lev][:], lo_b[lev - 1][:],
                                    Alu.mult, Alu.add)
                for bn in range(NBIN):
                    v.tensor_scalar(sink16[:], nv16[b][:], thr[:, bn:bn+1], 0.0,
                                    Alu.is_gt, Alu.add, accum_out=cge[:, bn:bn+1])
                cg_ps = pspool.tile([1, NBIN], F32, name="cg_ps", tag="pss")
                nc.tensor.matmul(cg_ps[:], ones_col[:], cge[:])
                v.tensor_copy(cget[:], cg_ps[:])
                v.tensor_scalar(gek[:], cget[:], kk[:], None, Alu.is_ge)
                v.tensor_reduce(out=scnt[:], in_=gek[:], axis=Ax.X, op=Alu.add)
                v.tensor_scalar(lo_new[:], scnt[:], 1.0, wl[lev][0:1, :],
                                Alu.subtract, Alu.mult)
                v.tensor_scalar(tau[lev][:], scnt[:], wl[lev][0:1, :], None, Alu.mult)
                if lev > 0:
                    v.tensor_tensor(out=lo_new[:], in0=lo_new[:],
                                    in1=lo_b[lev - 1][0:1, :], op=Alu.add)
                    v.tensor_tensor(out=tau[lev][:], in0=tau[lev][:],
                                    in1=lo_b[lev - 1][0:1, :], op=Alu.add)
                pbcast(lo_b[lev][:], lo_new[:])

            pbcast(tau_b[:], tau[NLEV - 1][:])
            v.tensor_scalar(s4[:], nv[b][:], tau_b[:], 0.0, Alu.is_gt,
                            Alu.add, accum_out=cnt_pp[:])
            v.tensor_tensor(out=s5[:], in0=nv[b][:], in1=s4[:], op=Alu.mult)
            v.tensor_reduce(out=sum_pp[:], in_=s5[:], axis=Ax.X, op=Alu.add)

            # ---------------- gather scalars ----------------
            v.tensor_copy(stack[:, 0:1], npp[:])
            v.tensor_copy(stack[:, 1:2], locsum_pp[:])
            v.tensor_copy(stack[:, 2:3], possum_pp[:])
            v.tensor_copy(stack[:, 3:4], cnt_pp[:])
            st_ps = pspool.tile([1, 4], F32, name="st_ps", tag="pss")
            nc.tensor.matmul(st_ps[:], ones_col[:], stack[:])
            sm_ps = pspool.tile([1, 1], F32, name="sm_ps", tag="pss")
            nc.tensor.matmul(sm_ps[:], ones_col[:], sum_pp[:])

            v.tensor_copy(res_sb[:, 0:4], st_ps[:])
            v.tensor_copy(res_sb[:, 4:5], sm_ps[:])
            v.tensor_copy(res_sb[:, 5:6], tau[NLEV - 1][:])
            v.tensor_copy(res_sb[:, 6:7], maxv1[:])
            v.tensor_copy(res_sb[:, 7:8], kk[:])
            nc.sync.dma_start(res_d[b], res_sb[:])

    nc.compile()
    return nc


_NC_CACHE = None


def _get_nc():
    global _NC_CACHE
    if _NC_CACHE is None:
        _NC_CACHE = _build_nc()
    return _NC_CACHE


def _make_in_maps(inputs):
    bbox_pred = np.asarray(inputs["bbox_pred"])
    conf_pred = np.asarray(inputs["conf_pred"])
    anchors = np.asarray(inputs["anchors"])
    gt_boxes = np.asarray(inputs["gt_boxes"])
    anch_h = np.ascontiguousarray(anchors.reshape(P, COLS * 4), dtype=np.float32)
    in_maps = []
    for i in range(NCORE):
        bsl = slice(IMG * i, IMG * (i + 1))
        in_maps.append({
            "anch": anch_h,
            "bbox": np.ascontiguousarray(
                bbox_pred[bsl].reshape(IMG, P, COLS * 4), dtype=np.float32),
            "conf": np.ascontiguousarray(
                conf_pred[bsl].reshape(IMG, P, COLS), dtype=np.float32),
            "gtb": np.ascontiguousarray(
                gt_boxes[bsl].reshape(IMG, 1, G * 4), dtype=np.float32),
        })
    return in_maps


def kernel(bbox_pred, conf_pred, anchors, gt_boxes):
    nc = _get_nc()
    in_maps = _make_in_maps(dict(bbox_pred=bbox_pred, conf_pred=conf_pred,
                                 anchors=anchors, gt_boxes=gt_boxes))
    out = run_bass_kernel_spmd(nc, in_maps, core_ids=list(range(NCORE)))

    loc_total = np.float32(0.0)
    conf_total = np.float32(0.0)
    npos_total = np.float32(0.0)
    for i in range(NCORE):
        res = out.results[i]["res"]  # [IMG, 1, 8]
        for b in range(IMG):
            npos, locsum, possum, cnt_gt, sum_gt, tau_hi, maxv, kdev = \
                [np.float32(x) for x in res[b, 0, :8]]
            k = np.float32(min(NEG_POS_RATIO * npos, A - npos))
            wl_last = np.float32(maxv / NBIN ** NLEV)
            rem = max(np.float32(0.0), np.float32(k - cnt_gt))
            neg = np.float32(sum_gt + rem * (tau_hi - wl_last * np.float32(0.5)))
            loc_total = np.float32(loc_total + locsum)
            conf_total = np.float32(conf_total + possum + neg)
            npos_total = np.float32(npos_total + npos)
    num_pos = np.float32(max(1.0, npos_total))
    loc_loss = np.float32(loc_total / num_pos)
    conf_loss = np.float32(conf_total / num_pos)
    return (np.float32(loc_loss + conf_loss), conf_loss, loc_loss)



# revision 4
# speedup vs baseline: 25.1913x; 25.1913x over previous
"""Trainium2 Bass kernel for nn_DetectionLoss (SSD-style detection loss).

Strategy: data-parallel over the batch — 16 images, 8 NeuronCores, 2 images
per core. Each core computes, per image: the dense [A, G] anchor/gt IoU grid
(anchors on partitions, gt along the free dim, supertiles of 16 anchor
columns x 32 gts = 512-wide vector ops), per-anchor best-gt matching
(max + is_equal one-hot, summed against gt coords), per-gt best-anchor
forcing (column max via PE transpose + second eq pass over the SBUF-resident
grid), DIoU loc loss, sigmoid focal conf loss, and hard-negative mining via
a two-level (64x64) threshold-count search that returns the exact
sum-above-threshold plus boundary-bin statistics. The host combines the
per-image scalars (sum / max(1, n_pos)) exactly as the reference does.
"""
import sys

sys.path.insert(0, '/opt/trn_rl_repo')

import numpy as np
import concourse.bass as bass
import concourse.bacc as bacc
import concourse.mybir as mybir
from concourse.tile import TileContext
from concourse.bass_utils import run_bass_kernel_spmd
from concourse.masks import make_identity
from contextlib import ExitStack

Alu = mybir.AluOpType
Act = mybir.ActivationFunctionType
Ax = mybir.AxisListType
F32 = mybir.dt.float32
I32 = mybir.dt.int32

P = 128
A = 65536
G = 32
IMG = 2            # images per core
NCORE = 8
COLS = A // P      # 512 anchor columns per partition
U = 32             # anchor columns per supertile
W = U * G          # 1024 free elems per supertile
NSUP = COLS // U   # 32 supertiles
EPS = 1e-7
NBIN = 16          # histogram bins per mining level
NLEV = 3           # mining levels (resolution NBIN**NLEV = 4096)
NEG_POS_RATIO = 3.0


def _build_nc():
    nc = bacc.Bacc("TRN2", target_bir_lowering=False, debug=False)
    anch_d = nc.dram_tensor("anch", [P, COLS * 4], F32, kind="ExternalInput")
    bbox_d = nc.dram_tensor("bbox", [IMG, P, COLS * 4], F32, kind="ExternalInput")
    conf_d = nc.dram_tensor("conf", [IMG, P, COLS], F32, kind="ExternalInput")
    gt_d = nc.dram_tensor("gtb", [IMG, 1, G * 4], F32, kind="ExternalInput")
    res_d = nc.dram_tensor("res", [IMG, 1, 8], F32, kind="ExternalOutput")

    v = nc.vector
    sc = nc.scalar
    pe = nc.tensor

    with TileContext(nc) as tc, ExitStack() as ctx:
        pool = ctx.enter_context(tc.tile_pool(name="main", bufs=1))
        pspool = ctx.enter_context(tc.tile_pool(name="ps", bufs=1, space="PSUM"))

        def T(name, cols, parts=P):
            return pool.tile([parts, cols], F32, name=name)

        # ---------------- static tiles ----------------
        anch_sb = T("anch_sb", COLS * 4)        # 8 KB/part
        grid = T("grid", COLS * G)              # 64 KB/part
        bbox_sb = T("bbox_sb", COLS * 4)        # 8 KB
        conf_sb = T("conf_sb", COLS)
        areaAe = T("areaAe", COLS)
        rowmax = T("rowmax", COLS)
        forced = T("forced", COLS)
        pos = T("pos", COLS)
        matched_il = T("matched_il", COLS * 4)  # 8 KB, (col, coord) interleaved
        colparts = T("colparts", NSUP * G)
        nv = [T(f"nv{i}", COLS) for i in range(IMG)]
        cl = T("cl", COLS)
        sink = T("sink", COLS)
        BF16 = mybir.dt.bfloat16
        FP16 = mybir.dt.float16
        nv16 = [pool.tile([P, COLS], FP16, name=f"nv16_{i}") for i in range(IMG)]
        sink16 = pool.tile([P, COLS], FP16, name="sink16")
        fb16 = pool.tile([P, W], BF16, name="fb16")
        # grid-phase scratch
        lt = T("lt", 2 * W)                     # 8 KB
        rb = T("rb", 2 * W)                     # 8 KB
        inter = T("inter", W)
        csum = T("csum", W)
        rec = T("rec", W)
        ismax = T("ismax", W)
        tsb = [T(f"tsb{i}", P) for i in range(2)]  # transposed ismax chunks
        gtmat = T("gtmat", 16)                  # block-diag gt coords [128,(uu,c)]
        # wide scratch (diou/focal), reused heavily
        w0 = T("w0", 2 * COLS)
        w1_ = T("w1_", 2 * COLS)
        s0 = T("s0", COLS)
        s1 = T("s1", COLS)
        s2 = T("s2", COLS)
        s3 = T("s3", COLS)
        s4 = T("s4", COLS)
        s5 = T("s5", COLS)
        # gt tiles
        GT_W = G * 4
        gtrow_sb = T("gtrow_sb", GT_W, parts=1)
        gt_il = T("gt_il", GT_W)
        gtc = [T(f"gtc{c}", G) for c in range(4)]
        sG = T("sG", G)
        glo = T("glo", 2 * G)
        ghi = T("ghi", 2 * G)
        cmb = T("cmb", G)
        colT = T("colT", G)
        ctt = T("ctt", P, parts=G)
        cmax_col = T("cmax_col", 1, parts=G)
        cm_row = T("cm_row", G, parts=1)
        mx_row = T("mx_row", P, parts=1)
        # small column tiles
        npp = T("npp", 1)
        locsum_pp = T("locsum_pp", 1)
        possum_pp = T("possum_pp", 1)
        cnt_pp = T("cnt_pp", 1)
        sum_pp = T("sum_pp", 1)
        maxv_pp = T("maxv_pp", 1)
        maxvb = T("maxvb", 1)
        w1c = T("w1c", 1)
        tau_b = T("tau_b", 1)
        stack = T("stack", 4)
        ones_col = T("ones_col", 1)
        ones_row = T("ones_row", P, parts=1)
        ident = T("ident", P)
        iota_f = T("iota_f", NBIN)
        thr = T("thr", NBIN)
        cge = T("cge", NBIN)
        wl = [T(f"wl{l}", 1) for l in range(NLEV)]
        lo_b = [T(f"lo_b{l}", 1) for l in range(NLEV)]
        cget = T("cget", NBIN, parts=1)
        gek = T("gek", NBIN, parts=1)
        scnt = T("scnt", 1, parts=1)
        lo_new = T("lo_new", 1, parts=1)
        tau = [T(f"tau{l}", 1, parts=1) for l in range(NLEV)]
        maxv1 = T("maxv1", 1, parts=1)
        npos1 = T("npos1", 1, parts=1)
        k1 = T("k1", 1, parts=1)
        k2 = T("k2", 1, parts=1)
        kk = T("kk", 1, parts=1)
        res_sb = T("res_sb", 8, parts=1)
        iota_i = pool.tile([P, NBIN], I32, name="iota_i")

        # ---------------- constants ----------------
        nc.sync.dma_start(anch_sb[:], anch_d[:])
        anch3 = anch_sb[:].rearrange("p (n c) -> p n c", c=4)
        v.tensor_tensor(out=s0[:], in0=anch3[:, :, 2:3].squeeze(2),
                        in1=anch3[:, :, 0:1].squeeze(2), op=Alu.subtract)
        v.tensor_tensor(out=s1[:], in0=anch3[:, :, 3:4].squeeze(2),
                        in1=anch3[:, :, 1:2].squeeze(2), op=Alu.subtract)
        v.tensor_tensor(out=areaAe[:], in0=s0[:], in1=s1[:], op=Alu.mult)
        v.tensor_scalar(areaAe[:], areaAe[:], float(EPS), None, Alu.add)

        v.memset(ones_col[:], 1.0)
        v.memset(ones_row[:], 1.0)
        make_identity(nc, ident[:])

        def pbcast(dst, src_row):
            """Broadcast a [1, n] partition-0 row to [P, n] via a K=1 matmul."""
            n = src_row.shape[-1]
            bc_ps = pspool.tile([P, G], F32, name="bc_ps", tag="pss")
            nc.tensor.matmul(bc_ps[:, 0:n], ones_row[:], src_row)
            v.tensor_copy(dst, bc_ps[:, 0:n])

        def pbcast_wide(dst, src_row, n):
            """Broadcast a [1, n] partition-0 row to [P, n] via a K=1 matmul."""
            bc_ps = pspool.tile([P, GT_W], F32, name="bcw_ps", tag="pssw")
            nc.tensor.matmul(bc_ps[:, 0:n], ones_row[:], src_row)
            v.tensor_copy(dst, bc_ps[:, 0:n])
        nc.gpsimd.iota(iota_i[:], pattern=[[1, NBIN]], base=0, channel_multiplier=0)
        v.tensor_copy(iota_f[:], iota_i[:])

        for b in range(IMG):
            # ---------------- per-image loads ----------------
            nc.sync.dma_start(bbox_sb[:], bbox_d[b])
            nc.sync.dma_start(conf_sb[:], conf_d[b])
            bb3 = bbox_sb[:].rearrange("p (n c) -> p n c", c=4)

            # Load gt as one tiny [1, 128] row; broadcast on-chip via PE.
            # (partition_broadcast DMA generates 4096 4-byte descriptors and
            # takes ~18us per call — the old version stalled the whole grid.)
            nc.sync.dma_start(gtrow_sb[:], gt_d[b])
            pbcast_wide(gt_il[:], gtrow_sb[:], GT_W)
            gt_il3 = gt_il[:].rearrange("p (g c) -> p g c", c=4)
            for c in range(4):
                v.tensor_copy(gtc[c][:], gt_il3[:, :, c:c+1].squeeze(2))
            # block-diagonal gt-coordinate matrix for the matched-gt matmul:
            # gtmat[uu*32+g, uu*4+c] = gt[g, c]
            v.memset(gtmat[:], 0.0)
            gt2d = gt_d[b].rearrange("q (g c) -> (q g) c", c=4)  # [32, 4] dram
            for uu in range(4):
                nc.sync.dma_start(gtmat[uu * G:(uu + 1) * G, uu * 4:(uu + 1) * 4],
                                  gt2d)
            v.tensor_tensor(out=glo[:, 0:G], in0=gtc[2][:], in1=gtc[0][:], op=Alu.subtract)
            v.tensor_tensor(out=glo[:, G:], in0=gtc[3][:], in1=gtc[1][:], op=Alu.subtract)
            v.tensor_tensor(out=sG[:], in0=glo[:, 0:G], in1=glo[:, G:], op=Alu.mult)
            v.tensor_copy(glo[:, 0:G], gtc[0][:])
            v.tensor_copy(glo[:, G:], gtc[1][:])
            v.tensor_copy(ghi[:, 0:G], gtc[2][:])
            v.tensor_copy(ghi[:, G:], gtc[3][:])

            def gb(t):  # [P, G] tile -> broadcast [P, U, G]
                return t[:].unsqueeze(1).to_broadcast([P, U, G])

            # ---------------- grid phase ----------------
            for s in range(NSUP):
                csl = slice(s * U, (s + 1) * U)
                lt4 = lt[:].rearrange("p (u c g) -> p u c g", c=2, g=G)
                rb4 = rb[:].rearrange("p (u c g) -> p u c g", c=2, g=G)
                a_lo = anch3[:, csl, 0:2].unsqueeze(3).to_broadcast([P, U, 2, G])
                a_hi = anch3[:, csl, 2:4].unsqueeze(3).to_broadcast([P, U, 2, G])
                g_lo = glo[:].rearrange("p (c g) -> p c g", g=G).unsqueeze(1).to_broadcast([P, U, 2, G])
                g_hi = ghi[:].rearrange("p (c g) -> p c g", g=G).unsqueeze(1).to_broadcast([P, U, 2, G])
                v.tensor_tensor(out=lt4, in0=a_lo, in1=g_lo, op=Alu.max)
                v.tensor_tensor(out=rb4, in0=a_hi, in1=g_hi, op=Alu.min)
                v.tensor_tensor(out=lt[:], in0=rb[:], in1=lt[:], op=Alu.subtract)
                sc.activation(lt[:], lt[:], Act.Relu)
                wr4 = lt[:].rearrange("p (u c g) -> p u c g", c=2, g=G)

                inter3 = inter[:].rearrange("p (u g) -> p u g", g=G)
                v.tensor_tensor(out=inter3, in0=wr4[:, :, 0, :], in1=wr4[:, :, 1, :],
                                op=Alu.mult)
                csum3 = csum[:].rearrange("p (u g) -> p u g", g=G)
                v.tensor_tensor(out=csum3,
                                in0=areaAe[:, csl].unsqueeze(2).to_broadcast([P, U, G]),
                                in1=gb(sG), op=Alu.add)
                v.tensor_tensor(out=csum[:], in0=csum[:], in1=inter[:], op=Alu.subtract)
                v.reciprocal_approx_fast(out=rec[:], in_=csum[:])

                gsl = grid[:, s * W:(s + 1) * W]
                g3 = gsl.rearrange("p (u g) -> p u g", g=G)
                v.tensor_tensor(out=g3, in0=inter[:].rearrange("p (u g) -> p u g", g=G),
                                in1=rec[:].rearrange("p (u g) -> p u g", g=G), op=Alu.mult)
                v.tensor_reduce(out=rowmax[:, csl], in_=g3, axis=Ax.X, op=Alu.max)
                ismax3 = ismax[:].rearrange("p (u g) -> p u g", g=G)
                v.tensor_tensor(out=ismax3, in0=g3,
                                in1=rowmax[:, csl].unsqueeze(2).to_broadcast([P, U, G]),
                                op=Alu.is_equal)
                # matched gt coords via PE: transpose each [128,128] ismax chunk,
                # then contract over gt with the block-diagonal gtmat.
                mout = pspool.tile([P, P], F32, name="mout", tag="mout")
                for j in range(W // P):
                    tp = pspool.tile([P, P], F32, name=f"tp{j % 2}", tag=f"tp{j % 2}")
                    pe.transpose(tp[:], ismax[:, j * P:(j + 1) * P], ident[:])
                    sc.copy(tsb[j % 2][:], tp[:])
                    nc.tensor.matmul(mout[:, j * 16:(j + 1) * 16], tsb[j % 2][:],
                                     gtmat[:])
                sc.copy(matched_il[:, s * (U * 4):(s + 1) * (U * 4)], mout[:])
                g3r = gsl.rearrange("p (u g) -> p g u", g=G)
                v.tensor_reduce(out=colparts[:, s * G:(s + 1) * G], in_=g3r,
                                axis=Ax.X, op=Alu.max)

            # ---------------- column max finalize ----------------
            v.tensor_reduce(out=colT[:],
                            in_=colparts[:].rearrange("p (s g) -> p g s", g=G),
                            axis=Ax.X, op=Alu.max)
            ct_ps = pspool.tile([G, P], F32, name="ct_ps", tag="pss")
            pe.transpose(ct_ps[:], colT[:], ident[:])
            v.tensor_copy(ctt[:], ct_ps[:])
            v.tensor_reduce(out=cmax_col[:], in_=ctt[:], axis=Ax.X, op=Alu.max)
            cm_ps = pspool.tile([1, G], F32, name="cm_ps", tag="pss")
            pe.transpose(cm_ps[:], cmax_col[:], ident[:G, :G])
            v.tensor_copy(cm_row[:], cm_ps[:])
            pbcast(cmb[:], cm_row[:])

            # ---------------- forced pass ----------------
            for s in range(NSUP):
                csl = slice(s * U, (s + 1) * U)
                g3 = grid[:, s * W:(s + 1) * W].rearrange("p (u g) -> p u g", g=G)
                fb3 = fb16[:].rearrange("p (u g) -> p u g", g=G)
                v.tensor_tensor(out=fb3, in0=g3, in1=gb(cmb), op=Alu.is_equal)
                v.tensor_reduce(out=forced[:, csl], in_=fb3, axis=Ax.X, op=Alu.max)

            # ---------------- pos / n_pos ----------------
            v.tensor_scalar(pos[:], rowmax[:], 0.5, None, Alu.is_gt)
            v.tensor_tensor(out=pos[:], in0=pos[:], in1=forced[:], op=Alu.max)
            v.tensor_reduce(out=npp[:], in_=pos[:], axis=Ax.X, op=Alu.add)

            # ---------------- DIoU loc loss ----------------
            def bc(c):
                return bb3[:, :, c:c+1].squeeze(2)

            m3 = matched_il[:].rearrange("p (n c) -> p n c", c=4)

            def mc(c):
                return m3[:, :, c:c+1].squeeze(2)

            # w0 = lt(x|y), w1_ = rb(x|y)
            v.tensor_tensor(out=w0[:, 0:COLS], in0=bc(0), in1=mc(0), op=Alu.max)
            v.tensor_tensor(out=w0[:, COLS:], in0=bc(1), in1=mc(1), op=Alu.max)
            v.tensor_tensor(out=w1_[:, 0:COLS], in0=bc(2), in1=mc(2), op=Alu.min)
            v.tensor_tensor(out=w1_[:, COLS:], in0=bc(3), in1=mc(3), op=Alu.min)
            v.tensor_tensor(out=w0[:], in0=w1_[:], in1=w0[:], op=Alu.subtract)
            sc.activation(w0[:], w0[:], Act.Relu)
            v.tensor_tensor(out=s0[:], in0=w0[:, 0:COLS], in1=w0[:, COLS:], op=Alu.mult)
            # s0 = inter; areas -> s1 (pred), s2 (matched)
            v.tensor_tensor(out=w0[:, 0:COLS], in0=bc(2), in1=bc(0), op=Alu.subtract)
            v.tensor_tensor(out=w0[:, COLS:], in0=bc(3), in1=bc(1), op=Alu.subtract)
            v.tensor_tensor(out=s1[:], in0=w0[:, 0:COLS], in1=w0[:, COLS:], op=Alu.mult)
            v.tensor_tensor(out=w0[:, 0:COLS], in0=mc(2), in1=mc(0), op=Alu.subtract)
            v.tensor_tensor(out=w0[:, COLS:], in0=mc(3), in1=mc(1), op=Alu.subtract)
            v.tensor_tensor(out=s2[:], in0=w0[:, 0:COLS], in1=w0[:, COLS:], op=Alu.mult)
            v.tensor_tensor(out=s1[:], in0=s1[:], in1=s2[:], op=Alu.add)
            v.tensor_tensor(out=s1[:], in0=s1[:], in1=s0[:], op=Alu.subtract)
            v.tensor_scalar(s1[:], s1[:], float(EPS), None, Alu.add)
            v.reciprocal_approx_accurate(out=s2[:], in_=s1[:], scratch=s3[:])
            v.tensor_tensor(out=s0[:], in0=s0[:], in1=s2[:], op=Alu.mult)  # s0 = iou
            # enclosing box
            v.tensor_tensor(out=w0[:, 0:COLS], in0=bc(0), in1=mc(0), op=Alu.min)
            v.tensor_tensor(out=w0[:, COLS:], in0=bc(1), in1=mc(1), op=Alu.min)
            v.tensor_tensor(out=w1_[:, 0:COLS], in0=bc(2), in1=mc(2), op=Alu.max)
            v.tensor_tensor(out=w1_[:, COLS:], in0=bc(3), in1=mc(3), op=Alu.max)
            v.tensor_tensor(out=w0[:], in0=w1_[:], in1=w0[:], op=Alu.subtract)
            sc.activation(w0[:], w0[:], Act.Square)
            v.tensor_tensor(out=s1[:], in0=w0[:, 0:COLS], in1=w0[:, COLS:], op=Alu.add)
            v.tensor_scalar(s1[:], s1[:], float(EPS), None, Alu.add)
            v.reciprocal_approx_accurate(out=s2[:], in_=s1[:], scratch=s3[:])  # 1/c2
            # center distance
            v.tensor_tensor(out=w0[:, 0:COLS], in0=bc(0), in1=bc(2), op=Alu.add)
            v.tensor_tensor(out=w0[:, COLS:], in0=bc(1), in1=bc(3), op=Alu.add)
            v.tensor_tensor(out=w1_[:, 0:COLS], in0=mc(0), in1=mc(2), op=Alu.add)
            v.tensor_tensor(out=w1_[:, COLS:], in0=mc(1), in1=mc(3), op=Alu.add)
            v.tensor_tensor(out=w0[:], in0=w0[:], in1=w1_[:], op=Alu.subtract)
            sc.activation(w0[:], w0[:], Act.Square, scale=0.5)
            v.tensor_tensor(out=s3[:], in0=w0[:, 0:COLS], in1=w0[:, COLS:], op=Alu.add)
            v.tensor_tensor(out=s3[:], in0=s3[:], in1=s2[:], op=Alu.mult)  # d2/c2
            v.tensor_scalar(s0[:], s0[:], -1.0, 1.0, Alu.mult, Alu.add)   # 1 - iou
            v.tensor_tensor(out=s3[:], in0=s3[:], in1=s0[:], op=Alu.add)
            v.tensor_scalar(s3[:], s3[:], 100.0, None, Alu.min)
            v.tensor_tensor(out=s4[:], in0=s3[:], in1=pos[:], op=Alu.mult)
            v.tensor_reduce(out=locsum_pp[:], in_=s4[:], axis=Ax.X, op=Alu.add)

            # ---------------- focal conf loss ----------------
            sc.activation(s0[:], conf_sb[:], Act.Sigmoid)
            sc.activation(s1[:], conf_sb[:], Act.Exp)
            sc.activation(s1[:], s1[:], Act.Ln, bias=1.0)
            v.tensor_tensor(out=s2[:], in0=conf_sb[:], in1=pos[:], op=Alu.mult)
            v.tensor_tensor(out=s2[:], in0=s1[:], in1=s2[:], op=Alu.subtract)  # ce
            v.tensor_scalar(s3[:], pos[:], -2.0, 1.0, Alu.mult, Alu.add)
            v.tensor_tensor(out=s3[:], in0=s0[:], in1=s3[:], op=Alu.mult)
            v.tensor_tensor(out=s3[:], in0=s3[:], in1=pos[:], op=Alu.add)  # 1-p_t
            sc.activation(s3[:], s3[:], Act.Square)
            v.tensor_tensor(out=cl[:], in0=s3[:], in1=s2[:], op=Alu.mult)
            v.tensor_scalar(s3[:], pos[:], -0.5, 0.75, Alu.mult, Alu.add)
            v.tensor_tensor(out=cl[:], in0=cl[:], in1=s3[:], op=Alu.mult)
            v.tensor_scalar(cl[:], cl[:], 100.0, None, Alu.min)
            v.tensor_tensor(out=s4[:], in0=cl[:], in1=pos[:], op=Alu.mult)
            v.tensor_reduce(out=possum_pp[:], in_=s4[:], axis=Ax.X, op=Alu.add)
            v.tensor_tensor(out=nv[b][:], in0=cl[:], in1=s4[:], op=Alu.subtract)
            v.tensor_copy(nv16[b][:], nv[b][:])

            # ---------------- hard negative mining ----------------
            v.tensor_reduce(out=maxv_pp[:], in_=nv[b][:], axis=Ax.X, op=Alu.max)
            mx_ps = pspool.tile([1, P], F32, name="mx_ps", tag="pss")
            pe.transpose(mx_ps[:], maxv_pp[:], ident[:])
            v.tensor_copy(mx_row[:], mx_ps[:])
            v.tensor_reduce(out=maxv1[:], in_=mx_row[:], axis=Ax.X, op=Alu.max)

            np_ps = pspool.tile([1, 1], F32, name="np_ps", tag="pss")
            nc.tensor.matmul(np_ps[:], ones_col[:], npp[:])
            v.tensor_copy(npos1[:], np_ps[:])
            v.tensor_scalar(k1[:], npos1[:], NEG_POS_RATIO, None, Alu.mult)
            v.tensor_scalar(k2[:], npos1[:], -1.0, float(A), Alu.mult, Alu.add)
            v.tensor_tensor(out=kk[:], in0=k1[:], in1=k2[:], op=Alu.min)

            pbcast(maxvb[:], maxv1[:])
            v.tensor_scalar(w1c[:], maxvb[:], 1.0 / NBIN, None, Alu.mult)

            for lev in range(NLEV):
                if lev == 0:
                    v.tensor_copy(wl[0][:], w1c[:])
                    v.tensor_scalar(thr[:], iota_f[:], wl[0][:], None, Alu.mult)
                else:
                    v.tensor_scalar(wl[lev][:], wl[lev - 1][:], 1.0 / NBIN, None,
                                    Alu.mult)
                    v.tensor_scalar(thr[:], iota_f[:], wl[lev][:], lo_b[lev - 1][:],
                                    Alu.mult, Alu.add)
                for bn in range(NBIN):
                    v.tensor_scalar(sink16[:], nv16[b][:], thr[:, bn:bn+1], 0.0,
                                    Alu.is_gt, Alu.add, accum_out=cge[:, bn:bn+1])
                cg_ps = pspool.tile([1, NBIN], F32, name="cg_ps", tag="pss")
                nc.tensor.matmul(cg_ps[:], ones_col[:], cge[:])
                v.tensor_copy(cget[:], cg_ps[:])
                v.tensor_scalar(gek[:], cget[:], kk[:], None, Alu.is_ge)
                v.tensor_reduce(out=scnt[:], in_=gek[:], axis=Ax.X, op=Alu.add)
                v.tensor_scalar(lo_new[:], scnt[:], 1.0, wl[lev][0:1, :],
                                Alu.subtract, Alu.mult)
                v.tensor_scalar(tau[lev][:], scnt[:], wl[lev][0:1, :], None, Alu.mult)
                if lev > 0:
                    v.tensor_tensor(out=lo_new[:], in0=lo_new[:],
                                    in1=lo_b[lev - 1][0:1, :], op=Alu.add)
                    v.tensor_tensor(out=tau[lev][:], in0=tau[lev][:],
                                    in1=lo_b[lev - 1][0:1, :], op=Alu.add)
                pbcast(lo_b[lev][:], lo_new[:])

            pbcast(tau_b[:], tau[NLEV - 1][:])
            v.tensor_scalar(s4[:], nv[b][:], tau_b[:], 0.0, Alu.is_gt,
                            Alu.add, accum_out=cnt_pp[:])
            v.tensor_tensor(out=s5[:], in0=nv[b][:], in1=s4[:], op=Alu.mult)
            v.tensor_reduce(out=sum_pp[:], in_=s5[:], axis=Ax.X, op=Alu.add)

            # ---------------- gather scalars ----------------
            v.tensor_copy(stack[:, 0:1], npp[:])
            v.tensor_copy(stack[:, 1:2], locsum_pp[:])
            v.tensor_copy(stack[:, 2:3], possum_pp[:])
            v.tensor_copy(stack[:, 3:4], cnt_pp[:])
            st_ps = pspool.tile([1, 4], F32, name="st_ps", tag="pss")
            nc.tensor.matmul(st_ps[:], ones_col[:], stack[:])
            sm_ps = pspool.tile([1, 1], F32, name="sm_ps", tag="pss")
            nc.tensor.matmul(sm_ps[:], ones_col[:], sum_pp[:])

            v.tensor_copy(res_sb[:, 0:4], st_ps[:])
            v.tensor_copy(res_sb[:, 4:5], sm_ps[:])
            v.tensor_copy(res_sb[:, 5:6], tau[NLEV - 1][:])
            v.tensor_copy(res_sb[:, 6:7], maxv1[:])
            v.tensor_copy(res_sb[:, 7:8], kk[:])
            nc.sync.dma_start(res_d[b], res_sb[:])

    nc.compile()
    return nc


_NC_CACHE = None


def _get_nc():
    global _NC_CACHE
    if _NC_CACHE is None:
        _NC_CACHE = _build_nc()
    return _NC_CACHE


def _make_in_maps(inputs):
    bbox_pred = np.asarray(inputs["bbox_pred"])
    conf_pred = np.asarray(inputs["conf_pred"])
    anchors = np.asarray(inputs["anchors"])
    gt_boxes = np.asarray(inputs["gt_boxes"])
    anch_h = np.ascontiguousarray(anchors.reshape(P, COLS * 4), dtype=np.float32)
    in_maps = []
    for i in range(NCORE):
        bsl = slice(IMG * i, IMG * (i + 1))
        in_maps.append({
            "anch": anch_h,
            "bbox": np.ascontiguousarray(
                bbox_pred[bsl].reshape(IMG, P, COLS * 4), dtype=np.float32),
            "conf": np.ascontiguousarray(
                conf_pred[bsl].reshape(IMG, P, COLS), dtype=np.float32),
            "gtb": np.ascontiguousarray(
                gt_boxes[bsl].reshape(IMG, 1, G * 4), dtype=np.float32),
        })
    return in_maps


def kernel(bbox_pred, conf_pred, anchors, gt_boxes):
    nc = _get_nc()
    in_maps = _make_in_maps(dict(bbox_pred=bbox_pred, conf_pred=conf_pred,
                                 anchors=anchors, gt_boxes=gt_boxes))
    out = run_bass_kernel_spmd(nc, in_maps, core_ids=list(range(NCORE)))

    loc_total = np.float32(0.0)
    conf_total = np.float32(0.0)
    npos_total = np.float32(0.0)
    for i in range(NCORE):
        res = out.results[i]["res"]  # [IMG, 1, 8]
        for b in range(IMG):
            npos, locsum, possum, cnt_gt, sum_gt, tau_hi, maxv, kdev = \
                [np.float32(x) for x in res[b, 0, :8]]
            k = np.float32(min(NEG_POS_RATIO * npos, A - npos))
            wl_last = np.float32(maxv / NBIN ** NLEV)
            rem = max(np.float32(0.0), np.float32(k - cnt_gt))
            neg = np.float32(sum_gt + rem * (tau_hi - wl_last * np.float32(0.5)))
            loc_total = np.float32(loc_total + locsum)
            conf_total = np.float32(conf_total + possum + neg)
            npos_total = np.float32(npos_total + npos)
    num_pos = np.float32(max(1.0, npos_total))
    loc_loss = np.float32(loc_total / num_pos)
    conf_loss = np.float32(conf_total / num_pos)
    return (np.float32(loc_loss + conf_loss), conf_loss, loc_loss)

